# revision 27
# baseline (speedup 1.0000x reference)
"""Trainium2 Bass kernel for nn_CMAModel (memory-augmented causal attention).

Sharding: 8 cores = 2 batches x 4 head-groups. Each core handles one batch and
4 heads (256 channels); the output projection is row-parallel and the 4
per-batch partials are summed on the host.

v2 design notes (all matmul operands bf16):
  - Score matmuls use zero-padded stationaries kTz [128, par, mq, S] so every
    matmul streams the full 128-partition array (par0 heads hold k on rows
    0:64 with rows 64:128 zeroed; par1 the reverse).  The moving operand is
    the stacked head-pair qT, whose other head multiplies the zero half.
  - PV matmuls take a 128-col window of V_s so LDWEIGHTS gets FWL and the
    full array stays active; junk output rows land in unused PSUM partitions.
    V layout per head parity: par0 [v(64)|ones], par1 [ones|v(64)], so par1
    heads emit Y on partitions 64:128 directly (no cross-partition DMA).
  - j-chunk pairs per head: one kT stationary serves score matmuls for both
    chunks; mem tiles processed mid-loop so the j0 combine overlaps the
    j1-only tail.
  - Score->exp->PV software-pipelined: PV of tile i is emitted after the
    score matmuls of tile i+1 so the PE streams while ACT runs exp.
  - Gate: sigmoid on ACT directly; per-head broadcast via
    gpsimd.partition_broadcast (no DRAM bounce).
  - Combine: Z row -> spread-DMA [1,512]->[128,4] -> recip -> gather ->
    partition_broadcast; Y = (Ac + g*Am) * (1/Z) on DVE.
  - conv + out-projection of chunk pair jp are interleaved into the
    attention stream of pair jp+1.
"""
import contextlib
import ctypes
import os
import sys
import types

import numpy as np

# ---------------------------------------------------------------- constants
B, T, C = 2, 2048, 1024
H, HD = 16, 64
M = 256
G = 4                 # head-groups (cores per batch)
HPG = H // G          # 4 heads per core
CPG = HPG * HD        # 256 channels per core
S = T + 2 * M         # 2560 kv rows
SM = 2 * M            # 512 memory rows
NKT = C // 128        # 8 contraction tiles
NST = S // 128        # 20 S tiles (16 chunk + 4 mem)
TC = 512              # T chunk size
NTC = T // TC         # 4
K = 4                 # conv taps
SCALE = 1.0 / float(np.sqrt(HD))
VW0 = (0, 2, 130, 132)  # per-head start col of the 128-wide V stationary

_MM_DTYPE = os.environ.get("BASS_MM_DTYPE", "bfloat16")

_BUILT = None


# ------------------------------------------------------- axon NTFF hook shim
def _install_ntff_hook():
    """The agent image lacks antenv.axon_hooks; synthesize it so
    run_bass_kernel_spmd(trace=True) can capture NTFF profiles."""
    if "antenv.axon_hooks" in sys.modules:
        return
    so_path = "/opt/axon/libaxon_pjrt.so"
    hook = None
    if os.path.exists(so_path):
        try:
            lib = ctypes.CDLL(so_path)
            if hasattr(lib, "axon_start_nrt_profile"):
                lib.axon_start_nrt_profile.argtypes = [
                    ctypes.POINTER(ctypes.c_int64),
                    ctypes.c_size_t,
                ]
                lib.axon_start_nrt_profile.restype = ctypes.c_int64
                lib.axon_stop_nrt_profile.argtypes = [ctypes.c_char_p]
                lib.axon_stop_nrt_profile.restype = ctypes.c_int64

                @contextlib.contextmanager
                def _hook(output_dir, device_ids):
                    import jax

                    jax.devices()
                    if device_ids:
                        ids = (ctypes.c_int64 * len(device_ids))(*device_ids)
                        rc = lib.axon_start_nrt_profile(ids, len(device_ids))
                    else:
                        rc = lib.axon_start_nrt_profile(None, 0)
                    if rc != 0:
                        raise RuntimeError(f"axon_start_nrt_profile rc={rc}")
                    try:
                        yield
                    finally:
                        n = lib.axon_stop_nrt_profile(str(output_dir).encode())
                        if n < 0:
                            raise RuntimeError(f"axon_stop_nrt_profile rc={n}")

                hook = _hook
        except OSError:
            pass
    mod = types.ModuleType("antenv.axon_hooks")
    mod.get_axon_ntff_profile_hook = lambda: hook
    mod.set_axon_ntff_profile_hook = lambda h: None
    sys.modules["antenv.axon_hooks"] = mod


# ------------------------------------------------------------- device build
def _build_program():
    import concourse.tile as tile
    from concourse import bacc, mybir
    from concourse.masks import make_upper_triangular

    f32 = mybir.dt.float32
    mdt = getattr(mybir.dt, _MM_DTYPE)

    nc = bacc.Bacc("TRN2", target_bir_lowering=False, debug=False, num_devices=8)

    xT = nc.dram_tensor("xT", [C, T], mdt, kind="ExternalInput").ap()
    memT = nc.dram_tensor("memT", [C, SM], mdt, kind="ExternalInput").ap()
    WqT = nc.dram_tensor("WqT", [C, CPG], mdt, kind="ExternalInput").ap()
    WkT = nc.dram_tensor("WkT", [C, CPG], mdt, kind="ExternalInput").ap()
    WvTa = nc.dram_tensor("WvTa", [C, 65 * HPG], mdt, kind="ExternalInput").ap()
    WgT = nc.dram_tensor("WgT", [C, 128], mdt, kind="ExternalInput").ap()
    gbn = nc.dram_tensor("gbn", [HPG, 1], f32, kind="ExternalInput").ap()
    WoT = nc.dram_tensor("WoT", [CPG, C], mdt, kind="ExternalInput").ap()
    cw = nc.dram_tensor("cw", [CPG, K], f32, kind="ExternalInput").ap()
    cb = nc.dram_tensor("cb", [CPG, 1], f32, kind="ExternalInput").ap()
    out = nc.dram_tensor("out", [T, C], f32, kind="ExternalOutput").ap()

    Exp = mybir.ActivationFunctionType.Exp
    Sig = mybir.ActivationFunctionType.Sigmoid
    AMULT = mybir.AluOpType.mult
    AADD = mybir.AluOpType.add

    with tile.TileContext(nc) as tc:
        with contextlib.ExitStack() as ctx:
            const = ctx.enter_context(tc.tile_pool(name="const", bufs=1))
            xpool = ctx.enter_context(tc.tile_pool(name="xpool", bufs=2))
            sb = ctx.enter_context(tc.tile_pool(name="sb", bufs=1))
            work = ctx.enter_context(tc.tile_pool(name="work", bufs=3))
            small = ctx.enter_context(tc.tile_pool(name="small", bufs=1))
            psum = ctx.enter_context(
                tc.tile_pool(name="psum", bufs=1, space="PSUM")
            )

            # ---- persistent activations (declare early for memsets)
            qT_s = sb.tile([128, 2, T], mdt)
            kTz = sb.tile([128, 2, 2, S], mdt)      # (par, mq)
            V_s = sb.tile([128, NST, 65 * HPG], mdt)
            gsig = sb.tile([HPG, T], mdt)
            gb = sb.tile([128, HPG, T], mdt)

            # zero halves of kTz once; projections only write the live half
            nc.gpsimd.memset(kTz[64:128, 0, :, :], 0.0)
            nc.gpsimd.memset(kTz[0:64, 1, :, :], 0.0)

            # ---- weights (k+mems first: memory projections start earliest)
            wk_s = const.tile([128, NKT, CPG], mdt)
            nc.sync.dma_start(out=wk_s, in_=WkT.rearrange("(a p) n -> p a n", p=128))
            mems = xpool.tile([128, NKT, SM], mdt, tag="xbig")
            nc.sync.dma_start(out=mems, in_=memT.rearrange("(a p) t -> p a t", p=128))
            wva_s = const.tile([128, NKT, 65 * HPG], mdt)
            nc.sync.dma_start(out=wva_s, in_=WvTa.rearrange("(a p) n -> p a n", p=128))
            wq_s = const.tile([128, NKT, CPG], mdt)
            nc.sync.dma_start(out=wq_s, in_=WqT.rearrange("(a p) n -> p a n", p=128))
            wg_s = const.tile([128, NKT, 128], mdt)
            nc.sync.dma_start(out=wg_s, in_=WgT.rearrange("(a p) n -> p a n", p=128))
            xh0 = xpool.tile([128, NKT, T // 2], mdt, tag="xbig")
            xTr = xT.rearrange("(a p) t -> p a t", p=128)
            for k in range(NKT):
                nc.sync.dma_start(out=xh0[:, k, :], in_=xTr[:, k, : T // 2])
            wo_s = const.tile([128, 2, C], mdt)
            nc.sync.dma_start(out=wo_s, in_=WoT.rearrange("(a p) n -> p a n", p=128))
            cw_s = const.tile([128, 2, K], f32)
            nc.sync.dma_start(out=cw_s, in_=cw.rearrange("(a p) n -> p a n", p=128))
            cb_s = const.tile([128, 2, 1], f32)
            nc.sync.dma_start(out=cb_s, in_=cb.rearrange("(a p) n -> p a n", p=128))
            gbn_s = const.tile([HPG, 1], f32)
            nc.sync.dma_start(out=gbn_s, in_=gbn)

            triz = const.tile([128, 128], mdt)
            make_upper_triangular(nc, triz, val=1.0, diag=True)

            def set_ones_cols(st):
                for c0 in (64, 194):
                    oc = V_s[:, st, c0:c0 + 2]
                    nc.vector.tensor_scalar(
                        oc, oc, 0.0, 1.0, AMULT, AADD
                    )

            # ---- memory k/v projections (first PE work)
            for mq in range(2):
                pk2 = psum.tile([128, 2, TC], f32, tag="ps", bufs=2)
                pk = pk2[:, 0, :]
                for k in range(NKT):
                    nc.tensor.matmul(
                        pk,
                        wk_s[:, k, mq * 128:(mq + 1) * 128],
                        mems[:, k, :],
                        start=(k == 0),
                        stop=(k == NKT - 1),
                    )
                nc.vector.tensor_copy(kTz[0:64, 0, mq, T:], pk[0:64])
                nc.vector.tensor_copy(kTz[64:128, 1, mq, T:], pk[64:128])
            for mt in range(SM // 128):
                st = 16 + mt
                pv2 = psum.tile([128, 2, TC], f32, tag="ps", bufs=2)
                pv = pv2[:, 0, 0:65 * HPG]
                for k in range(NKT):
                    nc.tensor.matmul(
                        pv,
                        mems[:, k, mt * 128:(mt + 1) * 128],
                        wva_s[:, k, :],
                        start=(k == 0),
                        stop=(k == NKT - 1),
                    )
                nc.vector.tensor_copy(V_s[:, st, :], pv)
                set_ones_cols(st)

            # ---- x projections: each chunk decomposes into matmul "groups"
            # (thunks) so late chunks can be interleaved into the
            # ACT-paced attention stream as PE filler work.
            def qk_group(xh, tglob, tloc, mq, which):
                def run():
                    ch = slice(tglob, tglob + TC)
                    w_s = wq_s if which == "q" else wk_s
                    p2 = psum.tile([128, 2, TC], f32, tag="ps", bufs=2)
                    p = p2[:, 0, :]
                    for k in range(NKT):
                        nc.tensor.matmul(
                            p,
                            w_s[:, k, mq * 128:(mq + 1) * 128],
                            xh[:, k, tloc:tloc + TC],
                            start=(k == 0),
                            stop=(k == NKT - 1),
                        )
                    if which == "q":
                        nc.vector.tensor_copy(qT_s[:, mq, ch], p)
                    else:
                        nc.vector.tensor_copy(kTz[0:64, 0, mq, ch], p[0:64])
                        nc.vector.tensor_copy(kTz[64:128, 1, mq, ch], p[64:128])
                return run

            def v_group(xh, tglob, tloc, mt):
                def run():
                    st = tglob // 128 + mt
                    pv2 = psum.tile([128, 2, TC], f32, tag="ps", bufs=2)
                    pv = pv2[:, 0, 0:65 * HPG]
                    for k in range(NKT):
                        nc.tensor.matmul(
                            pv,
                            xh[:, k, tloc + mt * 128:tloc + (mt + 1) * 128],
                            wva_s[:, k, :],
                            start=(k == 0),
                            stop=(k == NKT - 1),
                        )
                    nc.vector.tensor_copy(V_s[:, st, :], pv)
                    set_ones_cols(st)
                return run

            def gate_group(xh, tglob, tloc):
                def run():
                    ch = slice(tglob, tglob + TC)
                    pg2 = psum.tile([128, 2, TC], f32, tag="ps", bufs=2)
                    pg = pg2[:, 0, :]
                    for k in range(NKT):
                        nc.tensor.matmul(
                            pg,
                            wg_s[:, k, :],
                            xh[:, k, tloc:tloc + TC],
                            start=(k == 0),
                            stop=(k == NKT - 1),
                        )
                    nc.scalar.activation(
                        gsig[:, ch], pg[0:HPG, :], Sig, bias=gbn_s, scale=1.0
                    )
                    for hl in range(HPG):
                        g1 = small.tile([1, TC], mdt, tag="g1", bufs=2)
                        nc.sync.dma_start(out=g1, in_=gsig[hl:hl + 1, ch])
                        nc.gpsimd.partition_broadcast(
                            gb[:, hl, ch], g1, channels=128
                        )
                return run

            def chunk_groups(xh, tglob, tloc, with_gate=True):
                gs = []
                for mq in range(2):
                    gs.append(qk_group(xh, tglob, tloc, mq, "q"))
                    gs.append(qk_group(xh, tglob, tloc, mq, "k"))
                for mt in range(TC // 128):
                    gs.append(v_group(xh, tglob, tloc, mt))
                if with_gate:
                    gs.append(gate_group(xh, tglob, tloc))
                return gs

            for cn in range(2):
                for g_ in chunk_groups(xh0, cn * TC, cn * TC):
                    g_()
            xh1 = xpool.tile([128, NKT, T // 2], mdt, tag="xbig")
            for k in range(NKT):
                nc.sync.dma_start(out=xh1[:, k, :], in_=xTr[:, k, T // 2:])
            # chunks 2,3 become filler groups inside attention pair 0;
            # gate groups last so the two Sigmoids stay adjacent on ACT.
            fillers = (
                chunk_groups(xh1, T // 2, 0, with_gate=False)
                + chunk_groups(xh1, T // 2 + TC, TC, with_gate=False)
                + [gate_group(xh1, T // 2, 0), gate_group(xh1, T // 2 + TC, TC)]
            )
            fillers.reverse()  # pop() from the front

            attnout = xpool.tile([128, 4, T], mdt, tag="xbig")

            # ---- attention -----------------------------------------------
            # ---- flat attention schedule with a single software pipeline
            # spanning head boundaries: the PV matmuls of each score pair are
            # emitted after the NEXT pair's scores, and each head's combine
            # (+ interleaved outproj/conv) is deferred via a hook until its
            # last PV has been emitted.
            state = {"pend": None, "hook": None}

            def emit_pv_and_hook():
                if state["pend"] is not None:
                    Pt, cur, w0p = state["pend"]
                    state["pend"] = None
                    for dst, u, o, first, last, si in cur:
                        nc.tensor.matmul(
                            dst[:, o:],
                            V_s[:, si, w0p:w0p + 128],
                            Pt[:, u, o:],
                            start=first,
                            stop=last,
                        )
                if state["hook"] is not None:
                    h = state["hook"]
                    state["hook"] = None
                    h()

            def combine(hl, j, Ac, Am):
                """attnout Y rows for head hl = (Ac + g*Am) / Z."""
                mq, par = divmod(hl, 2)
                zr = 64 - par           # Z row within the A tiles
                ya = slice(64 * par, 64 * par + 64)
                ch = slice(TC * j, TC * (j + 1))
                # DVE partition offsets must be 32-aligned: sum the whole
                # 32-row block containing the Z row, DMA picks the row out.
                blk = slice((zr // 32) * 32, (zr // 32) * 32 + 32)
                zu = small.tile([128, TC], f32, tag="zu", bufs=2)
                nc.vector.tensor_copy(zu[blk, :], Ac[blk, :])
                nc.vector.tensor_add(zu[blk, :], zu[blk, :], Am[blk, :])
                zrg = small.tile([128, TC // 128], f32, tag="zrg", bufs=2)
                nc.sync.dma_start(out=zrg, in_=zu[zr:zr + 1, :])
                nc.vector.reciprocal(zrg, zrg)
                zt = small.tile([1, TC], f32, tag="zt", bufs=2)
                nc.sync.dma_start(out=zt, in_=zrg)
                zb = small.tile([128, TC], f32, tag="zb", bufs=2)
                nc.gpsimd.partition_broadcast(zb, zt, channels=128)
                t1 = small.tile([128, TC], f32, tag="t1", bufs=2)
                nc.vector.tensor_mul(t1[ya, :], Am[ya, :], gb[ya, hl, ch])
                nc.vector.tensor_add(t1[ya, :], t1[ya, :], Ac[ya, :])
                nc.vector.tensor_mul(attnout[ya, mq, ch], t1[ya, :], zb[ya, :])

            def conv_chunk(p, j):
                """Depthwise causal conv + residual + bias for chunk j."""
                c0 = TC * j
                y = attnout[:, p, :]
                R = attnout[:, 2 + p, :]
                nc.vector.tensor_scalar_add(
                    R[:, c0:c0 + TC], y[:, c0:c0 + TC], cb_s[:, p, :]
                )
                for k in range(K):
                    sh = K - 1 - k
                    a = c0 if (sh == 0 or c0 >= sh) else sh
                    nc.vector.scalar_tensor_tensor(
                        R[:, a:c0 + TC],
                        y[:, a - sh:c0 + TC - sh],
                        cw_s[:, p, k:k + 1],
                        R[:, a:c0 + TC],
                        AMULT,
                        AADD,
                    )

            def outproj_mt(mt):
                po = psum.tile([128, 2, TC], f32, tag="ps", bufs=2, name="po")
                for p in range(2):
                    stat = attnout[:, 2 + p, mt * 128:(mt + 1) * 128]
                    for nb in range(2):
                        nc.tensor.matmul(
                            po[:, nb, :], stat, wo_s[:, p, nb * TC:(nb + 1) * TC],
                            start=(p == 0), stop=(p == 1),
                        )
                ot = work.tile([128, 2, TC], f32, tag="ot", bufs=3)
                nc.vector.tensor_copy(ot, po)
                nc.sync.dma_start(
                    out=out[mt * 128:(mt + 1) * 128, :], in_=ot
                )

            pcnt = 0
            for j in range(NTC):
                for hl in range(HPG):
                    nct = 4 * (j + 1)
                    mq, par = divmod(hl, 2)
                    w0 = VW0[hl]
                    Ac = psum.tile([128, TC], f32, tag="pa", bufs=4, name="Ac")
                    Am = psum.tile([128, TC], f32, tag="pa", bufs=4, name="Am")
                    order = list(range(nct)) + [16, 17, 18, 19]
                    for pi in range(len(order) // 2):
                        pcnt += 1
                        if fillers and pcnt % 3 == 0:
                            emit_pv_and_hook()
                            fillers.pop()()
                        pair = order[2 * pi:2 * pi + 2]
                        ps = psum.tile([128, 2, TC], f32, tag="ps", bufs=2)
                        cur = []
                        for u, si in enumerate(pair):
                            is_mem = si >= 16
                            o = (
                                0 if (is_mem or si < 4 * j)
                                else 128 * si - TC * j
                            )
                            # scores full-width (cols [0:o) are junk the PV
                            # never reads) so the merged exp reads only
                            # freshly written PSUM
                            nc.tensor.matmul(
                                ps[:, u, :],
                                kTz[:, par, mq, si * 128:(si + 1) * 128],
                                qT_s[:, mq, TC * j:TC * (j + 1)],
                                start=True,
                                stop=True,
                            )
                            if is_mem:
                                dst, first, last = Am, si == 16, si == 19
                            else:
                                dst, first, last = Ac, si == 0, si == nct - 1
                            cur.append((dst, u, o, first, last, si))
                        Pt = work.tile([128, 2, TC], mdt, tag="P", bufs=3)
                        nc.scalar.activation(Pt, ps, Exp, scale=SCALE)
                        for dst, u, o, first, last, si in cur:
                            if (si < 16) and 4 * j <= si < 4 * j + 4:
                                nc.gpsimd.tensor_mul(
                                    Pt[:, u, o:o + 128],
                                    Pt[:, u, o:o + 128],
                                    triz,
                                )
                        emit_pv_and_hook()
                        state["pend"] = (Pt, cur, w0)

                    def mk_hook(j=j, hl=hl, Ac=Ac, Am=Am):
                        def h():
                            combine(hl, j, Ac, Am)
                            if hl == 1:
                                # heads 0,1 (pair p=0) done for chunk j
                                conv_chunk(0, j)
                            if hl == 3:
                                conv_chunk(1, j)
                            if j >= 1:
                                outproj_mt(4 * (j - 1) + hl)
                        return h

                    state["hook"] = mk_hook()
                if j == 1:
                    while fillers:
                        emit_pv_and_hook()
                        fillers.pop()()
            emit_pv_and_hook()
            for mt in range(12, 16):
                outproj_mt(mt)

    nc.compile()
    return nc


def _get_program():
    global _BUILT
    if _BUILT is None:
        _install_ntff_hook()
        _BUILT = _build_program()
    return _BUILT


# --------------------------------------------------------------- host side
def _tf32_round(a):
    """Cast to the matmul-operand dtype: TF32-round for float32r (data stays
    fp32 bits), bfloat16 for bf16 mode, passthrough for float32."""
    if _MM_DTYPE == "bfloat16":
        import ml_dtypes

        return np.ascontiguousarray(a, np.float32).astype(ml_dtypes.bfloat16)
    if _MM_DTYPE != "float32r":
        return np.ascontiguousarray(a, np.float32)
    u = np.ascontiguousarray(a, np.float32).view(np.uint32).astype(np.uint64)
    u = (u + 0x0FFF + ((u >> 13) & 1)) & np.uint64(0xFFFFE000)
    return u.astype(np.uint32).view(np.float32)


def host_prep(inputs):
    x = np.ascontiguousarray(np.asarray(inputs["x"], np.float32))
    fwd = np.asarray(inputs["fwd_mem"], np.float32)
    rev = np.asarray(inputs["rev_mem"], np.float32)
    Wq = np.asarray(inputs["Wq"], np.float32)
    Wk = np.asarray(inputs["Wk"], np.float32)
    Wv = np.asarray(inputs["Wv"], np.float32)
    Wo = np.asarray(inputs["Wo"], np.float32)
    gate_w = np.asarray(inputs["gate_w"], np.float32)
    gate_b = np.asarray(inputs["gate_b"], np.float32)
    canon_w = np.asarray(inputs["canon_w"], np.float32)
    canon_bias = np.asarray(inputs["canon_bias"], np.float32)

    Wg = (gate_w.astype(np.float64) @ Wq.astype(np.float64)).astype(np.float32)

    per_b, per_g = [], []
    for b in range(B):
        per_b.append({
            "xT": _tf32_round(x[b].T),
            "memT": _tf32_round(np.concatenate([fwd[b], rev[b]], axis=0).T),
        })
    for g in range(G):
        cs = slice(g * CPG, (g + 1) * CPG)
        # V layout: par0 heads [v(64)|ones], par1 heads [ones|v(64)];
        # ones cols written on device, zeros here.
        WvTa = np.zeros((C, 65 * HPG), np.float32)
        for h in range(HPG):
            rows = Wv[g * CPG + h * HD: g * CPG + (h + 1) * HD]
            c0 = 65 * h + (h % 2)
            WvTa[:, c0:c0 + 64] = rows.T
        hs = slice(g * HPG, (g + 1) * HPG)
        # gate stationary padded to 128 cols (junk repeats keep PE activity up)
        WgT = np.tile(Wg[hs].T, (1, 32))
        per_g.append({
            "WqT": _tf32_round(Wq[cs].T),
            "WkT": _tf32_round(Wk[cs].T),
            "WvTa": _tf32_round(WvTa),
            "WgT": _tf32_round(WgT),
            "gbn": np.ascontiguousarray(gate_b[hs]).reshape(HPG, 1),
            "WoT": _tf32_round(Wo[:, cs].T),
            "cw": np.ascontiguousarray(canon_w[cs, 0, :]),
            "cb": np.ascontiguousarray(canon_bias[cs]).reshape(CPG, 1),
        })
    return per_b, per_g


LAST_EXEC_NS = None
LAST_RESULTS = None


def kernel(**inputs):
    global LAST_EXEC_NS, LAST_RESULTS
    from concourse.bass_utils import run_bass_kernel_spmd

    nc = _get_program()
    per_b, per_g = host_prep(inputs)
    in_maps = []
    for core in range(8):
        b, g = divmod(core, G)
        m = {}
        m.update(per_b[b])
        m.update(per_g[g])
        in_maps.append(m)

    trace = bool(int(os.environ.get("KERNEL_TRACE", "0")))
    kw = {}
    if trace:
        tcores = os.environ.get("KERNEL_TRACE_CORES", "0")
        kw = dict(
            trace=True,
            trace_cores=[int(c) for c in tcores.split(",")],
            tmpdir=os.environ.get("KERNEL_TRACE_DIR", None),
        )
    res = run_bass_kernel_spmd(nc, in_maps, core_ids=list(range(8)), **kw)
    LAST_EXEC_NS = res.exec_time_ns
    LAST_RESULTS = res
    outp = np.zeros((B, T, C), np.float32)
    for core in range(8):
        b = core // G
        outp[b] += res.results[core]["out"]
    return outp


# revision 28
# speedup vs baseline: 1.7904x; 1.7904x over previous
"""Trainium2 Bass kernel for nn_CMAModel (memory-augmented causal attention).

Sharding: 8 cores = 2 batches x 4 head-groups. Each core handles one batch and
4 heads (256 channels); the output projection is row-parallel and the 4
per-batch partials are summed on the host.

v2 design notes (all matmul operands bf16):
  - Score matmuls use zero-padded stationaries kTz [128, par, mq, S] so every
    matmul streams the full 128-partition array (par0 heads hold k on rows
    0:64 with rows 64:128 zeroed; par1 the reverse).  The moving operand is
    the stacked head-pair qT, whose other head multiplies the zero half.
  - PV matmuls take a 128-col window of V_s so LDWEIGHTS gets FWL and the
    full array stays active; junk output rows land in unused PSUM partitions.
    V layout per head parity: par0 [v(64)|ones], par1 [ones|v(64)], so par1
    heads emit Y on partitions 64:128 directly (no cross-partition DMA).
  - j-chunk pairs per head: one kT stationary serves score matmuls for both
    chunks; mem tiles processed mid-loop so the j0 combine overlaps the
    j1-only tail.
  - Score->exp->PV software-pipelined: PV of tile i is emitted after the
    score matmuls of tile i+1 so the PE streams while ACT runs exp.
  - Gate: sigmoid on ACT directly; per-head broadcast via
    gpsimd.partition_broadcast (no DRAM bounce).
  - Combine: Z row -> spread-DMA [1,512]->[128,4] -> recip -> gather ->
    partition_broadcast; Y = (Ac + g*Am) * (1/Z) on DVE.
  - conv + out-projection of chunk pair jp are interleaved into the
    attention stream of pair jp+1.
"""
import contextlib
import ctypes
import os
import sys
import types

import numpy as np

# ---------------------------------------------------------------- constants
B, T, C = 2, 2048, 1024
H, HD = 16, 64
M = 256
G = 4                 # head-groups (cores per batch)
HPG = H // G          # 4 heads per core
CPG = HPG * HD        # 256 channels per core
S = T + 2 * M         # 2560 kv rows
SM = 2 * M            # 512 memory rows
NKT = C // 128        # 8 contraction tiles
NST = S // 128        # 20 S tiles (16 chunk + 4 mem)
TC = 512              # T chunk size
NTC = T // TC         # 4
K = 4                 # conv taps
SCALE = 1.0 / float(np.sqrt(HD))
VW0 = (0, 2, 130, 132)  # per-head start col of the 128-wide V stationary

_MM_DTYPE = os.environ.get("BASS_MM_DTYPE", "bfloat16")

_BUILT = None


# ------------------------------------------------------- axon NTFF hook shim
def _install_ntff_hook():
    """The agent image lacks antenv.axon_hooks; synthesize it so
    run_bass_kernel_spmd(trace=True) can capture NTFF profiles."""
    if "antenv.axon_hooks" in sys.modules:
        return
    so_path = "/opt/axon/libaxon_pjrt.so"
    hook = None
    if os.path.exists(so_path):
        try:
            lib = ctypes.CDLL(so_path)
            if hasattr(lib, "axon_start_nrt_profile"):
                lib.axon_start_nrt_profile.argtypes = [
                    ctypes.POINTER(ctypes.c_int64),
                    ctypes.c_size_t,
                ]
                lib.axon_start_nrt_profile.restype = ctypes.c_int64
                lib.axon_stop_nrt_profile.argtypes = [ctypes.c_char_p]
                lib.axon_stop_nrt_profile.restype = ctypes.c_int64

                @contextlib.contextmanager
                def _hook(output_dir, device_ids):
                    import jax

                    jax.devices()
                    if device_ids:
                        ids = (ctypes.c_int64 * len(device_ids))(*device_ids)
                        rc = lib.axon_start_nrt_profile(ids, len(device_ids))
                    else:
                        rc = lib.axon_start_nrt_profile(None, 0)
                    if rc != 0:
                        raise RuntimeError(f"axon_start_nrt_profile rc={rc}")
                    try:
                        yield
                    finally:
                        n = lib.axon_stop_nrt_profile(str(output_dir).encode())
                        if n < 0:
                            raise RuntimeError(f"axon_stop_nrt_profile rc={n}")

                hook = _hook
        except OSError:
            pass
    mod = types.ModuleType("antenv.axon_hooks")
    mod.get_axon_ntff_profile_hook = lambda: hook
    mod.set_axon_ntff_profile_hook = lambda h: None
    sys.modules["antenv.axon_hooks"] = mod


# ------------------------------------------------------------- device build
def _build_program():
    import concourse.tile as tile
    from concourse import bacc, mybir
    from concourse.masks import make_upper_triangular

    f32 = mybir.dt.float32
    mdt = getattr(mybir.dt, _MM_DTYPE)

    nc = bacc.Bacc("TRN2", target_bir_lowering=False, debug=False, num_devices=8)

    xT = nc.dram_tensor("xT", [C, T], mdt, kind="ExternalInput").ap()
    memT = nc.dram_tensor("memT", [C, SM], mdt, kind="ExternalInput").ap()
    WqT = nc.dram_tensor("WqT", [C, CPG], mdt, kind="ExternalInput").ap()
    WkT = nc.dram_tensor("WkT", [C, CPG], mdt, kind="ExternalInput").ap()
    WvTa = nc.dram_tensor("WvTa", [C, 65 * HPG], mdt, kind="ExternalInput").ap()
    WgT = nc.dram_tensor("WgT", [C, 128], mdt, kind="ExternalInput").ap()
    gbn = nc.dram_tensor("gbn", [HPG, 1], f32, kind="ExternalInput").ap()
    WoT = nc.dram_tensor("WoT", [CPG, C], mdt, kind="ExternalInput").ap()
    cw = nc.dram_tensor("cw", [CPG, K], f32, kind="ExternalInput").ap()
    cb = nc.dram_tensor("cb", [CPG, 1], f32, kind="ExternalInput").ap()
    out = nc.dram_tensor("out", [T, C], f32, kind="ExternalOutput").ap()

    Exp = mybir.ActivationFunctionType.Exp
    Sig = mybir.ActivationFunctionType.Sigmoid
    AMULT = mybir.AluOpType.mult
    AADD = mybir.AluOpType.add

    with tile.TileContext(nc) as tc:
        with contextlib.ExitStack() as ctx:
            const = ctx.enter_context(tc.tile_pool(name="const", bufs=1))
            xpool = ctx.enter_context(tc.tile_pool(name="xpool", bufs=2))
            sb = ctx.enter_context(tc.tile_pool(name="sb", bufs=1))
            work = ctx.enter_context(tc.tile_pool(name="work", bufs=3))
            small = ctx.enter_context(tc.tile_pool(name="small", bufs=1))
            psum = ctx.enter_context(
                tc.tile_pool(name="psum", bufs=1, space="PSUM")
            )

            # ---- persistent activations (declare early for memsets)
            qT_s = sb.tile([128, 2, T], mdt)
            kTz = sb.tile([128, 2, 2, S], mdt)      # (par, mq)
            V_s = sb.tile([128, NST, 65 * HPG], mdt)
            gsig = sb.tile([HPG, T], mdt)
            gb = sb.tile([128, HPG, T], mdt)

            # zero halves of kTz once; projections only write the live half
            nc.gpsimd.memset(kTz[64:128, 0, :, :], 0.0)
            nc.gpsimd.memset(kTz[0:64, 1, :, :], 0.0)

            # ---- weights (k+mems first: memory projections start earliest)
            wk_s = const.tile([128, NKT, CPG], mdt)
            nc.sync.dma_start(out=wk_s, in_=WkT.rearrange("(a p) n -> p a n", p=128))
            mems = xpool.tile([128, NKT, SM], mdt, tag="xbig")
            nc.sync.dma_start(out=mems, in_=memT.rearrange("(a p) t -> p a t", p=128))
            wva_s = const.tile([128, NKT, 65 * HPG], mdt)
            nc.sync.dma_start(out=wva_s, in_=WvTa.rearrange("(a p) n -> p a n", p=128))
            wq_s = const.tile([128, NKT, CPG], mdt)
            nc.sync.dma_start(out=wq_s, in_=WqT.rearrange("(a p) n -> p a n", p=128))
            wg_s = const.tile([128, NKT, 128], mdt)
            nc.sync.dma_start(out=wg_s, in_=WgT.rearrange("(a p) n -> p a n", p=128))
            xh0 = xpool.tile([128, NKT, T // 2], mdt, tag="xbig")
            xTr = xT.rearrange("(a p) t -> p a t", p=128)
            for k in range(NKT):
                nc.sync.dma_start(out=xh0[:, k, :], in_=xTr[:, k, : T // 2])
            wo_s = const.tile([128, 2, C], mdt)
            nc.sync.dma_start(out=wo_s, in_=WoT.rearrange("(a p) n -> p a n", p=128))
            cw_s = const.tile([128, 2, K], f32)
            nc.sync.dma_start(out=cw_s, in_=cw.rearrange("(a p) n -> p a n", p=128))
            cb_s = const.tile([128, 2, 1], f32)
            nc.sync.dma_start(out=cb_s, in_=cb.rearrange("(a p) n -> p a n", p=128))
            gbn_s = const.tile([HPG, 1], f32)
            nc.sync.dma_start(out=gbn_s, in_=gbn)

            triz = const.tile([128, 128], mdt)
            make_upper_triangular(nc, triz, val=1.0, diag=True)

            def set_ones_cols(st):
                for c0 in (64, 194):
                    oc = V_s[:, st, c0:c0 + 2]
                    nc.vector.tensor_scalar(
                        oc, oc, 0.0, 1.0, AMULT, AADD
                    )

            # ---- memory k/v projections (first PE work)
            for mq in range(2):
                pk2 = psum.tile([128, 2, TC], f32, tag="ps", bufs=2)
                pk = pk2[:, 0, :]
                for k in range(NKT):
                    nc.tensor.matmul(
                        pk,
                        wk_s[:, k, mq * 128:(mq + 1) * 128],
                        mems[:, k, :],
                        start=(k == 0),
                        stop=(k == NKT - 1),
                    )
                nc.vector.tensor_copy(kTz[0:64, 0, mq, T:], pk[0:64])
                nc.vector.tensor_copy(kTz[64:128, 1, mq, T:], pk[64:128])
            for mt in range(SM // 128):
                st = 16 + mt
                pv2 = psum.tile([128, 2, TC], f32, tag="ps", bufs=2)
                pv = pv2[:, 0, 0:65 * HPG]
                for k in range(NKT):
                    nc.tensor.matmul(
                        pv,
                        mems[:, k, mt * 128:(mt + 1) * 128],
                        wva_s[:, k, :],
                        start=(k == 0),
                        stop=(k == NKT - 1),
                    )
                nc.vector.tensor_copy(V_s[:, st, :], pv)
                set_ones_cols(st)

            # ---- x projections: each chunk decomposes into matmul "groups"
            # (thunks) so late chunks can be interleaved into the
            # ACT-paced attention stream as PE filler work.
            def qk_group(xh, tglob, tloc, mq, which):
                def run():
                    ch = slice(tglob, tglob + TC)
                    w_s = wq_s if which == "q" else wk_s
                    p2 = psum.tile([128, 2, TC], f32, tag="ps", bufs=2)
                    p = p2[:, 0, :]
                    for k in range(NKT):
                        nc.tensor.matmul(
                            p,
                            w_s[:, k, mq * 128:(mq + 1) * 128],
                            xh[:, k, tloc:tloc + TC],
                            start=(k == 0),
                            stop=(k == NKT - 1),
                        )
                    if which == "q":
                        nc.vector.tensor_copy(qT_s[:, mq, ch], p)
                    else:
                        nc.vector.tensor_copy(kTz[0:64, 0, mq, ch], p[0:64])
                        nc.vector.tensor_copy(kTz[64:128, 1, mq, ch], p[64:128])
                return run

            def v_group(xh, tglob, tloc, mt):
                def run():
                    st = tglob // 128 + mt
                    pv2 = psum.tile([128, 2, TC], f32, tag="ps", bufs=2)
                    pv = pv2[:, 0, 0:65 * HPG]
                    for k in range(NKT):
                        nc.tensor.matmul(
                            pv,
                            xh[:, k, tloc + mt * 128:tloc + (mt + 1) * 128],
                            wva_s[:, k, :],
                            start=(k == 0),
                            stop=(k == NKT - 1),
                        )
                    nc.vector.tensor_copy(V_s[:, st, :], pv)
                    set_ones_cols(st)
                return run

            def gate_group(xh, tglob, tloc):
                def run():
                    ch = slice(tglob, tglob + TC)
                    pg2 = psum.tile([128, 2, TC], f32, tag="ps", bufs=2)
                    pg = pg2[:, 0, :]
                    for k in range(NKT):
                        nc.tensor.matmul(
                            pg,
                            wg_s[:, k, :],
                            xh[:, k, tloc:tloc + TC],
                            start=(k == 0),
                            stop=(k == NKT - 1),
                        )
                    nc.scalar.activation(
                        gsig[:, ch], pg[0:HPG, :], Sig, bias=gbn_s, scale=1.0
                    )
                    for hl in range(HPG):
                        g1 = small.tile([1, TC], mdt, tag="g1", bufs=2)
                        nc.sync.dma_start(out=g1, in_=gsig[hl:hl + 1, ch])
                        nc.gpsimd.partition_broadcast(
                            gb[:, hl, ch], g1, channels=128
                        )
                return run

            def chunk_groups(xh, tglob, tloc, with_gate=True):
                gs = []
                for mq in range(2):
                    gs.append(qk_group(xh, tglob, tloc, mq, "q"))
                    gs.append(qk_group(xh, tglob, tloc, mq, "k"))
                for mt in range(TC // 128):
                    gs.append(v_group(xh, tglob, tloc, mt))
                if with_gate:
                    gs.append(gate_group(xh, tglob, tloc))
                return gs

            for cn in range(2):
                for g_ in chunk_groups(xh0, cn * TC, cn * TC):
                    g_()
            xh1 = xpool.tile([128, NKT, T // 2], mdt, tag="xbig")
            for k in range(NKT):
                nc.sync.dma_start(out=xh1[:, k, :], in_=xTr[:, k, T // 2:])
            # chunks 2,3 become filler groups inside attention pair 0;
            # gate groups last so the two Sigmoids stay adjacent on ACT.
            fillers = (
                chunk_groups(xh1, T // 2, 0, with_gate=False)
                + chunk_groups(xh1, T // 2 + TC, TC, with_gate=False)
                + [gate_group(xh1, T // 2, 0), gate_group(xh1, T // 2 + TC, TC)]
            )
            fillers.reverse()  # pop() from the front

            attnout = xpool.tile([128, 4, T], mdt, tag="xbig")

            # ---- attention -----------------------------------------------
            # ---- flat attention schedule with a single software pipeline
            # spanning head boundaries: the PV matmuls of each score pair are
            # emitted after the NEXT pair's scores, and each head's combine
            # (+ interleaved outproj/conv) is deferred via a hook until its
            # last PV has been emitted.
            state = {"pend": None, "hook": None}

            def emit_pv_and_hook():
                if state["pend"] is not None:
                    Pt, cur, w0p = state["pend"]
                    state["pend"] = None
                    for dst, u, o, first, last, si in cur:
                        nc.tensor.matmul(
                            dst[:, o:],
                            V_s[:, si, w0p:w0p + 128],
                            Pt[:, u, o:],
                            start=first,
                            stop=last,
                        )
                if state["hook"] is not None:
                    h = state["hook"]
                    state["hook"] = None
                    h()

            def combine(hl, j, Ac, Am):
                """attnout Y rows for head hl = (Ac + g*Am) / Z."""
                mq, par = divmod(hl, 2)
                zr = 64 - par           # Z row within the A tiles
                ya = slice(64 * par, 64 * par + 64)
                ch = slice(TC * j, TC * (j + 1))
                # DVE partition offsets must be 32-aligned: sum the whole
                # 32-row block containing the Z row, DMA picks the row out.
                blk = slice((zr // 32) * 32, (zr // 32) * 32 + 32)
                zu = small.tile([128, TC], f32, tag="zu", bufs=2)
                nc.vector.tensor_copy(zu[blk, :], Ac[blk, :])
                nc.vector.tensor_add(zu[blk, :], zu[blk, :], Am[blk, :])
                zrg = small.tile([128, TC // 128], f32, tag="zrg", bufs=2)
                nc.sync.dma_start(out=zrg, in_=zu[zr:zr + 1, :])
                nc.vector.reciprocal(zrg, zrg)
                zt = small.tile([1, TC], f32, tag="zt", bufs=2)
                nc.sync.dma_start(out=zt, in_=zrg)
                zb = small.tile([128, TC], f32, tag="zb", bufs=2)
                nc.gpsimd.partition_broadcast(zb, zt, channels=128)
                t1 = small.tile([128, TC], f32, tag="t1", bufs=2)
                nc.vector.tensor_mul(t1[ya, :], Am[ya, :], gb[ya, hl, ch])
                nc.vector.tensor_add(t1[ya, :], t1[ya, :], Ac[ya, :])
                nc.vector.tensor_mul(attnout[ya, mq, ch], t1[ya, :], zb[ya, :])

            def conv_chunk(p, j):
                """Depthwise causal conv + residual + bias for chunk j."""
                c0 = TC * j
                y = attnout[:, p, :]
                R = attnout[:, 2 + p, :]
                nc.vector.tensor_scalar_add(
                    R[:, c0:c0 + TC], y[:, c0:c0 + TC], cb_s[:, p, :]
                )
                for k in range(K):
                    sh = K - 1 - k
                    a = c0 if (sh == 0 or c0 >= sh) else sh
                    nc.vector.scalar_tensor_tensor(
                        R[:, a:c0 + TC],
                        y[:, a - sh:c0 + TC - sh],
                        cw_s[:, p, k:k + 1],
                        R[:, a:c0 + TC],
                        AMULT,
                        AADD,
                    )

            def outproj_mt(mt):
                po = psum.tile([128, 2, TC], f32, tag="ps", bufs=2, name="po")
                for p in range(2):
                    stat = attnout[:, 2 + p, mt * 128:(mt + 1) * 128]
                    for nb in range(2):
                        nc.tensor.matmul(
                            po[:, nb, :], stat, wo_s[:, p, nb * TC:(nb + 1) * TC],
                            start=(p == 0), stop=(p == 1),
                        )
                ot = work.tile([128, 2, TC], f32, tag="ot", bufs=3)
                nc.vector.tensor_copy(ot, po)
                nc.sync.dma_start(
                    out=out[mt * 128:(mt + 1) * 128, :], in_=ot
                )

            pcnt = 0
            for j in range(NTC):
                for hl in range(HPG):
                    nct = 4 * (j + 1)
                    mq, par = divmod(hl, 2)
                    w0 = VW0[hl]
                    Ac = psum.tile([128, TC], f32, tag="pa", bufs=4, name="Ac")
                    Am = psum.tile([128, TC], f32, tag="pa", bufs=4, name="Am")
                    order = list(range(nct)) + [16, 17, 18, 19]
                    for pi in range(len(order) // 2):
                        pcnt += 1
                        if fillers and pcnt % 3 == 0:
                            emit_pv_and_hook()
                            fillers.pop()()
                        pair = order[2 * pi:2 * pi + 2]
                        ps = psum.tile([128, 2, TC], f32, tag="ps", bufs=2)
                        cur = []
                        for u, si in enumerate(pair):
                            is_mem = si >= 16
                            o = (
                                0 if (is_mem or si < 4 * j)
                                else 128 * si - TC * j
                            )
                            # scores full-width (cols [0:o) are junk the PV
                            # never reads) so the merged exp reads only
                            # freshly written PSUM
                            nc.tensor.matmul(
                                ps[:, u, :],
                                kTz[:, par, mq, si * 128:(si + 1) * 128],
                                qT_s[:, mq, TC * j:TC * (j + 1)],
                                start=True,
                                stop=True,
                            )
                            if is_mem:
                                dst, first, last = Am, si == 16, si == 19
                            else:
                                dst, first, last = Ac, si == 0, si == nct - 1
                            cur.append((dst, u, o, first, last, si))
                        Pt = work.tile([128, 2, TC], mdt, tag="P", bufs=3)
                        nc.scalar.activation(Pt, ps, Exp, scale=SCALE)
                        for dst, u, o, first, last, si in cur:
                            if (si < 16) and 4 * j <= si < 4 * j + 4:
                                nc.vector.tensor_mul(
                                    Pt[:, u, o:o + 128],
                                    Pt[:, u, o:o + 128],
                                    triz,
                                )
                        emit_pv_and_hook()
                        state["pend"] = (Pt, cur, w0)

                    def mk_hook(j=j, hl=hl, Ac=Ac, Am=Am):
                        def h():
                            combine(hl, j, Ac, Am)
                            if hl == 1:
                                # heads 0,1 (pair p=0) done for chunk j
                                conv_chunk(0, j)
                            if hl == 3:
                                conv_chunk(1, j)
                            if j >= 1:
                                outproj_mt(4 * (j - 1) + hl)
                        return h

                    state["hook"] = mk_hook()
                if j == 1:
                    while fillers:
                        emit_pv_and_hook()
                        fillers.pop()()
            emit_pv_and_hook()
            for mt in range(12, 16):
                outproj_mt(mt)

    nc.compile()
    return nc


def _get_program():
    global _BUILT
    if _BUILT is None:
        _install_ntff_hook()
        _BUILT = _build_program()
    return _BUILT


# --------------------------------------------------------------- host side
def _tf32_round(a):
    """Cast to the matmul-operand dtype: TF32-round for float32r (data stays
    fp32 bits), bfloat16 for bf16 mode, passthrough for float32."""
    if _MM_DTYPE == "bfloat16":
        import ml_dtypes

        return np.ascontiguousarray(a, np.float32).astype(ml_dtypes.bfloat16)
    if _MM_DTYPE != "float32r":
        return np.ascontiguousarray(a, np.float32)
    u = np.ascontiguousarray(a, np.float32).view(np.uint32).astype(np.uint64)
    u = (u + 0x0FFF + ((u >> 13) & 1)) & np.uint64(0xFFFFE000)
    return u.astype(np.uint32).view(np.float32)


def host_prep(inputs):
    x = np.ascontiguousarray(np.asarray(inputs["x"], np.float32))
    fwd = np.asarray(inputs["fwd_mem"], np.float32)
    rev = np.asarray(inputs["rev_mem"], np.float32)
    Wq = np.asarray(inputs["Wq"], np.float32)
    Wk = np.asarray(inputs["Wk"], np.float32)
    Wv = np.asarray(inputs["Wv"], np.float32)
    Wo = np.asarray(inputs["Wo"], np.float32)
    gate_w = np.asarray(inputs["gate_w"], np.float32)
    gate_b = np.asarray(inputs["gate_b"], np.float32)
    canon_w = np.asarray(inputs["canon_w"], np.float32)
    canon_bias = np.asarray(inputs["canon_bias"], np.float32)

    Wg = (gate_w.astype(np.float64) @ Wq.astype(np.float64)).astype(np.float32)

    per_b, per_g = [], []
    for b in range(B):
        per_b.append({
            "xT": _tf32_round(x[b].T),
            "memT": _tf32_round(np.concatenate([fwd[b], rev[b]], axis=0).T),
        })
    for g in range(G):
        cs = slice(g * CPG, (g + 1) * CPG)
        # V layout: par0 heads [v(64)|ones], par1 heads [ones|v(64)];
        # ones cols written on device, zeros here.
        WvTa = np.zeros((C, 65 * HPG), np.float32)
        for h in range(HPG):
            rows = Wv[g * CPG + h * HD: g * CPG + (h + 1) * HD]
            c0 = 65 * h + (h % 2)
            WvTa[:, c0:c0 + 64] = rows.T
        hs = slice(g * HPG, (g + 1) * HPG)
        # gate stationary padded to 128 cols (junk repeats keep PE activity up)
        WgT = np.tile(Wg[hs].T, (1, 32))
        per_g.append({
            "WqT": _tf32_round(Wq[cs].T),
            "WkT": _tf32_round(Wk[cs].T),
            "WvTa": _tf32_round(WvTa),
            "WgT": _tf32_round(WgT),
            "gbn": np.ascontiguousarray(gate_b[hs]).reshape(HPG, 1),
            "WoT": _tf32_round(Wo[:, cs].T),
            "cw": np.ascontiguousarray(canon_w[cs, 0, :]),
            "cb": np.ascontiguousarray(canon_bias[cs]).reshape(CPG, 1),
        })
    return per_b, per_g


LAST_EXEC_NS = None
LAST_RESULTS = None


def kernel(**inputs):
    global LAST_EXEC_NS, LAST_RESULTS
    from concourse.bass_utils import run_bass_kernel_spmd

    nc = _get_program()
    per_b, per_g = host_prep(inputs)
    in_maps = []
    for core in range(8):
        b, g = divmod(core, G)
        m = {}
        m.update(per_b[b])
        m.update(per_g[g])
        in_maps.append(m)

    trace = bool(int(os.environ.get("KERNEL_TRACE", "0")))
    kw = {}
    if trace:
        tcores = os.environ.get("KERNEL_TRACE_CORES", "0")
        kw = dict(
            trace=True,
            trace_cores=[int(c) for c in tcores.split(",")],
            tmpdir=os.environ.get("KERNEL_TRACE_DIR", None),
        )
    res = run_bass_kernel_spmd(nc, in_maps, core_ids=list(range(8)), **kw)
    LAST_EXEC_NS = res.exec_time_ns
    LAST_RESULTS = res
    outp = np.zeros((B, T, C), np.float32)
    for core in range(8):
        b = core // G
        outp[b] += res.results[core]["out"]
    return outp


# revision 30
# speedup vs baseline: 2.1120x; 1.1796x over previous
"""Trainium2 Bass kernel for nn_CMAModel (memory-augmented causal attention).

Sharding: 8 cores = 2 batches x 4 head-groups. Each core handles one batch and
4 heads (256 channels); the output projection is row-parallel and the 4
per-batch partials are summed on the host.

v2 design notes (all matmul operands bf16):
  - Score matmuls use zero-padded stationaries kTz [128, par, mq, S] so every
    matmul streams the full 128-partition array (par0 heads hold k on rows
    0:64 with rows 64:128 zeroed; par1 the reverse).  The moving operand is
    the stacked head-pair qT, whose other head multiplies the zero half.
  - PV matmuls take a 128-col window of V_s so LDWEIGHTS gets FWL and the
    full array stays active; junk output rows land in unused PSUM partitions.
    V layout per head parity: par0 [v(64)|ones], par1 [ones|v(64)], so par1
    heads emit Y on partitions 64:128 directly (no cross-partition DMA).
  - j-chunk pairs per head: one kT stationary serves score matmuls for both
    chunks; mem tiles processed mid-loop so the j0 combine overlaps the
    j1-only tail.
  - Score->exp->PV software-pipelined: PV of tile i is emitted after the
    score matmuls of tile i+1 so the PE streams while ACT runs exp.
  - Gate: sigmoid on ACT directly; per-head broadcast via
    gpsimd.partition_broadcast (no DRAM bounce).
  - Combine: Z row -> spread-DMA [1,512]->[128,4] -> recip -> gather ->
    partition_broadcast; Y = (Ac + g*Am) * (1/Z) on DVE.
  - conv + out-projection of chunk pair jp are interleaved into the
    attention stream of pair jp+1.
"""
import contextlib
import ctypes
import os
import sys
import types

import numpy as np

# ---------------------------------------------------------------- constants
B, T, C = 2, 2048, 1024
H, HD = 16, 64
M = 256
G = 4                 # head-groups (cores per batch)
HPG = H // G          # 4 heads per core
CPG = HPG * HD        # 256 channels per core
S = T + 2 * M         # 2560 kv rows
SM = 2 * M            # 512 memory rows
NKT = C // 128        # 8 contraction tiles
NST = S // 128        # 20 S tiles (16 chunk + 4 mem)
TC = 512              # T chunk size
NTC = T // TC         # 4
K = 4                 # conv taps
SCALE = 1.0 / float(np.sqrt(HD))
VW0 = (0, 2, 130, 132)  # per-head start col of the 128-wide V stationary

_MM_DTYPE = os.environ.get("BASS_MM_DTYPE", "bfloat16")

_BUILT = None


# ------------------------------------------------------- axon NTFF hook shim
def _install_ntff_hook():
    """The agent image lacks antenv.axon_hooks; synthesize it so
    run_bass_kernel_spmd(trace=True) can capture NTFF profiles."""
    if "antenv.axon_hooks" in sys.modules:
        return
    so_path = "/opt/axon/libaxon_pjrt.so"
    hook = None
    if os.path.exists(so_path):
        try:
            lib = ctypes.CDLL(so_path)
            if hasattr(lib, "axon_start_nrt_profile"):
                lib.axon_start_nrt_profile.argtypes = [
                    ctypes.POINTER(ctypes.c_int64),
                    ctypes.c_size_t,
                ]
                lib.axon_start_nrt_profile.restype = ctypes.c_int64
                lib.axon_stop_nrt_profile.argtypes = [ctypes.c_char_p]
                lib.axon_stop_nrt_profile.restype = ctypes.c_int64

                @contextlib.contextmanager
                def _hook(output_dir, device_ids):
                    import jax

                    jax.devices()
                    if device_ids:
                        ids = (ctypes.c_int64 * len(device_ids))(*device_ids)
                        rc = lib.axon_start_nrt_profile(ids, len(device_ids))
                    else:
                        rc = lib.axon_start_nrt_profile(None, 0)
                    if rc != 0:
                        raise RuntimeError(f"axon_start_nrt_profile rc={rc}")
                    try:
                        yield
                    finally:
                        n = lib.axon_stop_nrt_profile(str(output_dir).encode())
                        if n < 0:
                            raise RuntimeError(f"axon_stop_nrt_profile rc={n}")

                hook = _hook
        except OSError:
            pass
    mod = types.ModuleType("antenv.axon_hooks")
    mod.get_axon_ntff_profile_hook = lambda: hook
    mod.set_axon_ntff_profile_hook = lambda h: None
    sys.modules["antenv.axon_hooks"] = mod


# ------------------------------------------------------------- device build
def _build_program():
    import concourse.tile as tile
    from concourse import bacc, mybir
    from concourse.masks import make_upper_triangular

    f32 = mybir.dt.float32
    mdt = getattr(mybir.dt, _MM_DTYPE)

    nc = bacc.Bacc("TRN2", target_bir_lowering=False, debug=False, num_devices=8)

    xT = nc.dram_tensor("xT", [C, T], mdt, kind="ExternalInput").ap()
    memT = nc.dram_tensor("memT", [C, SM], mdt, kind="ExternalInput").ap()
    WqT = nc.dram_tensor("WqT", [C, CPG], mdt, kind="ExternalInput").ap()
    WkT = nc.dram_tensor("WkT", [C, CPG], mdt, kind="ExternalInput").ap()
    WvTa = nc.dram_tensor("WvTa", [C, 65 * HPG], mdt, kind="ExternalInput").ap()
    WgT = nc.dram_tensor("WgT", [C, 128], mdt, kind="ExternalInput").ap()
    gbn = nc.dram_tensor("gbn", [HPG, 1], f32, kind="ExternalInput").ap()
    WoT = nc.dram_tensor("WoT", [CPG, C], mdt, kind="ExternalInput").ap()
    cw = nc.dram_tensor("cw", [CPG, K], f32, kind="ExternalInput").ap()
    cb = nc.dram_tensor("cb", [CPG, 1], f32, kind="ExternalInput").ap()
    out = nc.dram_tensor("out", [T, C], f32, kind="ExternalOutput").ap()

    Exp = mybir.ActivationFunctionType.Exp
    Sig = mybir.ActivationFunctionType.Sigmoid
    AMULT = mybir.AluOpType.mult
    AADD = mybir.AluOpType.add

    with tile.TileContext(nc) as tc:
        with contextlib.ExitStack() as ctx:
            const = ctx.enter_context(tc.tile_pool(name="const", bufs=1))
            xpool = ctx.enter_context(tc.tile_pool(name="xpool", bufs=2))
            sb = ctx.enter_context(tc.tile_pool(name="sb", bufs=1))
            work = ctx.enter_context(tc.tile_pool(name="work", bufs=3))
            small = ctx.enter_context(tc.tile_pool(name="small", bufs=1))
            psum = ctx.enter_context(
                tc.tile_pool(name="psum", bufs=1, space="PSUM")
            )

            # ---- persistent activations (declare early for memsets)
            qT_s = sb.tile([128, 2, T], mdt)
            kTz = sb.tile([128, 2, 2, S], mdt)      # (par, mq)
            V_s = sb.tile([128, NST, 65 * HPG], mdt)
            gsig = sb.tile([HPG, T], mdt)
            gb = sb.tile([128, HPG, T], mdt)

            # zero halves of kTz once; projections only write the live half
            nc.gpsimd.memset(kTz[64:128, 0, :, :], 0.0)
            nc.gpsimd.memset(kTz[0:64, 1, :, :], 0.0)

            # ---- weights (k+mems first: memory projections start earliest)
            wk_s = const.tile([128, NKT, CPG], mdt)
            nc.sync.dma_start(out=wk_s, in_=WkT.rearrange("(a p) n -> p a n", p=128))
            mems = xpool.tile([128, NKT, SM], mdt, tag="xbig")
            nc.sync.dma_start(out=mems, in_=memT.rearrange("(a p) t -> p a t", p=128))
            wva_s = const.tile([128, NKT, 65 * HPG], mdt)
            nc.sync.dma_start(out=wva_s, in_=WvTa.rearrange("(a p) n -> p a n", p=128))
            wq_s = const.tile([128, NKT, CPG], mdt)
            nc.sync.dma_start(out=wq_s, in_=WqT.rearrange("(a p) n -> p a n", p=128))
            wg_s = const.tile([128, NKT, 128], mdt)
            nc.sync.dma_start(out=wg_s, in_=WgT.rearrange("(a p) n -> p a n", p=128))
            xh0 = xpool.tile([128, NKT, T // 2], mdt, tag="xbig")
            xTr = xT.rearrange("(a p) t -> p a t", p=128)
            for k in range(NKT):
                nc.sync.dma_start(out=xh0[:, k, :], in_=xTr[:, k, : T // 2])
            wo_s = const.tile([128, 2, C], mdt)
            nc.sync.dma_start(out=wo_s, in_=WoT.rearrange("(a p) n -> p a n", p=128))
            cw_s = const.tile([128, 2, K], f32)
            nc.sync.dma_start(out=cw_s, in_=cw.rearrange("(a p) n -> p a n", p=128))
            cb_s = const.tile([128, 2, 1], f32)
            nc.sync.dma_start(out=cb_s, in_=cb.rearrange("(a p) n -> p a n", p=128))
            gbn_s = const.tile([HPG, 1], f32)
            nc.sync.dma_start(out=gbn_s, in_=gbn)

            triz = const.tile([128, 128], mdt)
            make_upper_triangular(nc, triz, val=1.0, diag=True)

            def set_ones_cols(st):
                for c0 in (64, 194):
                    oc = V_s[:, st, c0:c0 + 2]
                    nc.vector.tensor_scalar(
                        oc, oc, 0.0, 1.0, AMULT, AADD
                    )

            # ---- memory k/v projections (first PE work)
            for mq in range(2):
                pk2 = psum.tile([128, 2, TC], f32, tag="ps", bufs=2)
                pk = pk2[:, 0, :]
                for k in range(NKT):
                    nc.tensor.matmul(
                        pk,
                        wk_s[:, k, mq * 128:(mq + 1) * 128],
                        mems[:, k, :],
                        start=(k == 0),
                        stop=(k == NKT - 1),
                    )
                nc.vector.tensor_copy(kTz[0:64, 0, mq, T:], pk[0:64])
                nc.vector.tensor_copy(kTz[64:128, 1, mq, T:], pk[64:128])
            for mt in range(SM // 128):
                st = 16 + mt
                pv2 = psum.tile([128, 2, TC], f32, tag="ps", bufs=2)
                pv = pv2[:, 0, 0:65 * HPG]
                for k in range(NKT):
                    nc.tensor.matmul(
                        pv,
                        mems[:, k, mt * 128:(mt + 1) * 128],
                        wva_s[:, k, :],
                        start=(k == 0),
                        stop=(k == NKT - 1),
                    )
                nc.vector.tensor_copy(V_s[:, st, :], pv)
                set_ones_cols(st)

            # ---- x projections: each chunk decomposes into matmul "groups"
            # (thunks) so late chunks can be interleaved into the
            # ACT-paced attention stream as PE filler work.
            def qk_group(xh, tglob, tloc, mq, which):
                def run():
                    ch = slice(tglob, tglob + TC)
                    w_s = wq_s if which == "q" else wk_s
                    p2 = psum.tile([128, 2, TC], f32, tag="ps", bufs=2)
                    p = p2[:, 0, :]
                    for k in range(NKT):
                        nc.tensor.matmul(
                            p,
                            w_s[:, k, mq * 128:(mq + 1) * 128],
                            xh[:, k, tloc:tloc + TC],
                            start=(k == 0),
                            stop=(k == NKT - 1),
                        )
                    if which == "q":
                        nc.vector.tensor_copy(qT_s[:, mq, ch], p)
                    else:
                        nc.vector.tensor_copy(kTz[0:64, 0, mq, ch], p[0:64])
                        nc.vector.tensor_copy(kTz[64:128, 1, mq, ch], p[64:128])
                return run

            def v_group(xh, tglob, tloc, mt):
                def run():
                    st = tglob // 128 + mt
                    pv2 = psum.tile([128, 2, TC], f32, tag="ps", bufs=2)
                    pv = pv2[:, 0, 0:65 * HPG]
                    for k in range(NKT):
                        nc.tensor.matmul(
                            pv,
                            xh[:, k, tloc + mt * 128:tloc + (mt + 1) * 128],
                            wva_s[:, k, :],
                            start=(k == 0),
                            stop=(k == NKT - 1),
                        )
                    nc.vector.tensor_copy(V_s[:, st, :], pv)
                    set_ones_cols(st)
                return run

            def gate_group(xh, tglob, tloc):
                def run():
                    ch = slice(tglob, tglob + TC)
                    pg2 = psum.tile([128, 2, TC], f32, tag="ps", bufs=2)
                    pg = pg2[:, 0, :]
                    for k in range(NKT):
                        nc.tensor.matmul(
                            pg,
                            wg_s[:, k, :],
                            xh[:, k, tloc:tloc + TC],
                            start=(k == 0),
                            stop=(k == NKT - 1),
                        )
                    nc.scalar.activation(
                        gsig[:, ch], pg[0:HPG, :], Sig, bias=gbn_s, scale=1.0
                    )
                    for hl in range(HPG):
                        g1 = small.tile([1, TC], mdt, tag="g1", bufs=2)
                        nc.sync.dma_start(out=g1, in_=gsig[hl:hl + 1, ch])
                        nc.gpsimd.partition_broadcast(
                            gb[:, hl, ch], g1, channels=128
                        )
                return run

            def chunk_groups(xh, tglob, tloc, with_gate=True):
                gs = []
                for mq in range(2):
                    gs.append(qk_group(xh, tglob, tloc, mq, "q"))
                    gs.append(qk_group(xh, tglob, tloc, mq, "k"))
                for mt in range(TC // 128):
                    gs.append(v_group(xh, tglob, tloc, mt))
                if with_gate:
                    gs.append(gate_group(xh, tglob, tloc))
                return gs

            for cn in range(2):
                for g_ in chunk_groups(xh0, cn * TC, cn * TC):
                    g_()
            xh1 = xpool.tile([128, NKT, T // 2], mdt, tag="xbig")
            for k in range(NKT):
                nc.sync.dma_start(out=xh1[:, k, :], in_=xTr[:, k, T // 2:])
            # chunks 2,3 become filler groups inside attention pair 0;
            # gate groups last so the two Sigmoids stay adjacent on ACT.
            fillers = (
                chunk_groups(xh1, T // 2, 0, with_gate=False)
                + chunk_groups(xh1, T // 2 + TC, TC, with_gate=False)
                + [gate_group(xh1, T // 2, 0), gate_group(xh1, T // 2 + TC, TC)]
            )
            fillers.reverse()  # pop() from the front

            attnout = xpool.tile([128, 4, T], mdt, tag="xbig")

            # ---- attention -----------------------------------------------
            # ---- attention: two heads run as interleaved streams so the
            # PE computes one head's scores while ACT exps the other's.
            # Per stream, PV of pair k-1 is emitted after the scores of
            # pair k (software pipeline).

            def mk_head(j, hl):
                nct = 4 * (j + 1)
                mq, par = divmod(hl, 2)
                return {
                    "j": j, "hl": hl, "nct": nct, "mq": mq, "par": par,
                    "w0": VW0[hl],
                    "Ac": psum.tile([128, TC], f32, tag="pa", bufs=4, name="Ac"),
                    "Am": psum.tile([128, TC], f32, tag="pa", bufs=4, name="Am"),
                    "order": list(range(nct)) + [16, 17, 18, 19],
                    "pend": None,
                }

            def head_scores(Sd, pi):
                j, mq, par, nct = Sd["j"], Sd["mq"], Sd["par"], Sd["nct"]
                pair = Sd["order"][2 * pi:2 * pi + 2]
                ps = psum.tile([128, 2, TC], f32, tag="ps", bufs=2)
                cur = []
                for u, si in enumerate(pair):
                    is_mem = si >= 16
                    o = 0 if (is_mem or si < 4 * j) else 128 * si - TC * j
                    # scores full-width (cols [0:o) are junk the PV never
                    # reads) so the merged exp reads only fresh PSUM
                    nc.tensor.matmul(
                        ps[:, u, :],
                        kTz[:, par, mq, si * 128:(si + 1) * 128],
                        qT_s[:, mq, TC * j:TC * (j + 1)],
                        start=True,
                        stop=True,
                    )
                    if is_mem:
                        dst, first, last = Sd["Am"], si == 16, si == 19
                    else:
                        dst, first, last = Sd["Ac"], si == 0, si == nct - 1
                    cur.append((dst, u, o, first, last, si))
                Pt = work.tile([128, 2, TC], mdt, tag="P", bufs=4)
                nc.scalar.activation(Pt, ps, Exp, scale=SCALE)
                for dst, u, o, first, last, si in cur:
                    if (si < 16) and 4 * j <= si < 4 * j + 4:
                        nc.vector.tensor_mul(
                            Pt[:, u, o:o + 128], Pt[:, u, o:o + 128], triz
                        )
                return (Pt, cur)

            def emit_pv(Sd):
                if Sd["pend"] is None:
                    return
                Pt, cur = Sd["pend"]
                Sd["pend"] = None
                w0 = Sd["w0"]
                for dst, u, o, first, last, si in cur:
                    nc.tensor.matmul(
                        dst[:, o:],
                        V_s[:, si, w0:w0 + 128],
                        Pt[:, u, o:],
                        start=first,
                        stop=last,
                    )

            def combine(hl, j, Ac, Am):
                """attnout Y rows for head hl = (Ac + g*Am) / Z."""
                mq, par = divmod(hl, 2)
                zr = 64 - par           # Z row within the A tiles
                ya = slice(64 * par, 64 * par + 64)
                ch = slice(TC * j, TC * (j + 1))
                # DVE partition offsets must be 32-aligned: sum the whole
                # 32-row block containing the Z row, DMA picks the row out.
                blk = slice((zr // 32) * 32, (zr // 32) * 32 + 32)
                zu = small.tile([128, TC], f32, tag="zu", bufs=2)
                nc.vector.tensor_copy(zu[blk, :], Ac[blk, :])
                nc.vector.tensor_add(zu[blk, :], zu[blk, :], Am[blk, :])
                zrg = small.tile([128, TC // 128], f32, tag="zrg", bufs=2)
                nc.sync.dma_start(out=zrg, in_=zu[zr:zr + 1, :])
                nc.vector.reciprocal(zrg, zrg)
                zt = small.tile([1, TC], f32, tag="zt", bufs=2)
                nc.sync.dma_start(out=zt, in_=zrg)
                zb = small.tile([128, TC], f32, tag="zb", bufs=2)
                nc.gpsimd.partition_broadcast(zb, zt, channels=128)
                t1 = small.tile([128, TC], f32, tag="t1", bufs=2)
                nc.vector.tensor_mul(t1[ya, :], Am[ya, :], gb[ya, hl, ch])
                nc.vector.tensor_add(t1[ya, :], t1[ya, :], Ac[ya, :])
                nc.vector.tensor_mul(
                    attnout[ya, mq, ch], t1[ya, :], zb[ya, :]
                )

            def conv_chunk(p, j):
                """Depthwise causal conv + residual + bias for chunk j."""
                c0 = TC * j
                y = attnout[:, p, :]
                R = attnout[:, 2 + p, :]
                nc.vector.tensor_scalar_add(
                    R[:, c0:c0 + TC], y[:, c0:c0 + TC], cb_s[:, p, :]
                )
                for k in range(K):
                    sh = K - 1 - k
                    a = c0 if (sh == 0 or c0 >= sh) else sh
                    nc.vector.scalar_tensor_tensor(
                        R[:, a:c0 + TC],
                        y[:, a - sh:c0 + TC - sh],
                        cw_s[:, p, k:k + 1],
                        R[:, a:c0 + TC],
                        AMULT,
                        AADD,
                    )

            def outproj_mt(mt):
                po = psum.tile([128, 2, TC], f32, tag="ps", bufs=2, name="po")
                for p in range(2):
                    stat = attnout[:, 2 + p, mt * 128:(mt + 1) * 128]
                    for nb in range(2):
                        nc.tensor.matmul(
                            po[:, nb, :], stat,
                            wo_s[:, p, nb * TC:(nb + 1) * TC],
                            start=(p == 0), stop=(p == 1),
                        )
                ot = work.tile([128, 2, TC], f32, tag="ot", bufs=3)
                nc.vector.tensor_copy(ot, po)
                nc.sync.dma_start(
                    out=out[mt * 128:(mt + 1) * 128, :], in_=ot
                )

            pcnt = 0
            for j in range(NTC):
                for zi, (ha, hb) in enumerate(((0, 1), (2, 3))):
                    A = mk_head(j, ha)
                    Bh = mk_head(j, hb)
                    npairs = len(A["order"]) // 2
                    for pi in range(npairs):
                        for Sd in (A, Bh):
                            new = head_scores(Sd, pi)
                            emit_pv(Sd)
                            Sd["pend"] = new
                            pcnt += 1
                            if fillers and pcnt % 3 == 0:
                                fillers.pop()()
                    for Sd in (A, Bh):
                        emit_pv(Sd)
                        combine(Sd["hl"], j, Sd["Ac"], Sd["Am"])
                    # conv for head-pair p once both its heads are combined
                    conv_chunk(zi, j)
                    # previous chunk's output projection, two blocks per zip
                    if j >= 1:
                        outproj_mt(4 * (j - 1) + 2 * zi)
                        outproj_mt(4 * (j - 1) + 2 * zi + 1)
                if j == 1:
                    while fillers:
                        fillers.pop()()
            for mt in range(12, 16):
                outproj_mt(mt)

    nc.compile()
    return nc


def _get_program():
    global _BUILT
    if _BUILT is None:
        _install_ntff_hook()
        _BUILT = _build_program()
    return _BUILT


# --------------------------------------------------------------- host side
def _tf32_round(a):
    """Cast to the matmul-operand dtype: TF32-round for float32r (data stays
    fp32 bits), bfloat16 for bf16 mode, passthrough for float32."""
    if _MM_DTYPE == "bfloat16":
        import ml_dtypes

        return np.ascontiguousarray(a, np.float32).astype(ml_dtypes.bfloat16)
    if _MM_DTYPE != "float32r":
        return np.ascontiguousarray(a, np.float32)
    u = np.ascontiguousarray(a, np.float32).view(np.uint32).astype(np.uint64)
    u = (u + 0x0FFF + ((u >> 13) & 1)) & np.uint64(0xFFFFE000)
    return u.astype(np.uint32).view(np.float32)


def host_prep(inputs):
    x = np.ascontiguousarray(np.asarray(inputs["x"], np.float32))
    fwd = np.asarray(inputs["fwd_mem"], np.float32)
    rev = np.asarray(inputs["rev_mem"], np.float32)
    Wq = np.asarray(inputs["Wq"], np.float32)
    Wk = np.asarray(inputs["Wk"], np.float32)
    Wv = np.asarray(inputs["Wv"], np.float32)
    Wo = np.asarray(inputs["Wo"], np.float32)
    gate_w = np.asarray(inputs["gate_w"], np.float32)
    gate_b = np.asarray(inputs["gate_b"], np.float32)
    canon_w = np.asarray(inputs["canon_w"], np.float32)
    canon_bias = np.asarray(inputs["canon_bias"], np.float32)

    Wg = (gate_w.astype(np.float64) @ Wq.astype(np.float64)).astype(np.float32)

    per_b, per_g = [], []
    for b in range(B):
        per_b.append({
            "xT": _tf32_round(x[b].T),
            "memT": _tf32_round(np.concatenate([fwd[b], rev[b]], axis=0).T),
        })
    for g in range(G):
        cs = slice(g * CPG, (g + 1) * CPG)
        # V layout: par0 heads [v(64)|ones], par1 heads [ones|v(64)];
        # ones cols written on device, zeros here.
        WvTa = np.zeros((C, 65 * HPG), np.float32)
        for h in range(HPG):
            rows = Wv[g * CPG + h * HD: g * CPG + (h + 1) * HD]
            c0 = 65 * h + (h % 2)
            WvTa[:, c0:c0 + 64] = rows.T
        hs = slice(g * HPG, (g + 1) * HPG)
        # gate stationary padded to 128 cols (junk repeats keep PE activity up)
        WgT = np.tile(Wg[hs].T, (1, 32))
        per_g.append({
            "WqT": _tf32_round(Wq[cs].T),
            "WkT": _tf32_round(Wk[cs].T),
            "WvTa": _tf32_round(WvTa),
            "WgT": _tf32_round(WgT),
            "gbn": np.ascontiguousarray(gate_b[hs]).reshape(HPG, 1),
            "WoT": _tf32_round(Wo[:, cs].T),
            "cw": np.ascontiguousarray(canon_w[cs, 0, :]),
            "cb": np.ascontiguousarray(canon_bias[cs]).reshape(CPG, 1),
        })
    return per_b, per_g


LAST_EXEC_NS = None
LAST_RESULTS = None


def kernel(**inputs):
    global LAST_EXEC_NS, LAST_RESULTS
    from concourse.bass_utils import run_bass_kernel_spmd

    nc = _get_program()
    per_b, per_g = host_prep(inputs)
    in_maps = []
    for core in range(8):
        b, g = divmod(core, G)
        m = {}
        m.update(per_b[b])
        m.update(per_g[g])
        in_maps.append(m)

    trace = bool(int(os.environ.get("KERNEL_TRACE", "0")))
    kw = {}
    if trace:
        tcores = os.environ.get("KERNEL_TRACE_CORES", "0")
        kw = dict(
            trace=True,
            trace_cores=[int(c) for c in tcores.split(",")],
            tmpdir=os.environ.get("KERNEL_TRACE_DIR", None),
        )
    res = run_bass_kernel_spmd(nc, in_maps, core_ids=list(range(8)), **kw)
    LAST_EXEC_NS = res.exec_time_ns
    LAST_RESULTS = res
    outp = np.zeros((B, T, C), np.float32)
    for core in range(8):
        b = core // G
        outp[b] += res.results[core]["out"]
    return outp
